# revision 11
# baseline (speedup 1.0000x reference)
"""A3TGCN v2: chunk-major staircase ELL, single mixed grid per half.

Math (H0 == 0 in the reference => R-gate dead, weights foldable):
  Y   = A_hat @ X          X = x as [N, 48] (time-major), one sparse agg
  1-Z = sigmoid(y @ -Mz - cz)        (negation folded into weights)
  H_t = tanh  (y @ Mh + ch)
  out = relu(sum_t p_t (1-Z_t) H_t) @ Wout + bout

Structure per core (6272 dests):
  dests sorted by (-max(a,b), -(a>b), -min(a,b)) -> cols of 128.
  Per col j: KA_j/KB_j = cross-core max #edges from table half A/B.
  Cols grouped into chunks balanced by block count. Per chunk: one
  dma_gather per half (col-major staircase layout), DVE mult into bf16
  tmp, per-col strided reduce -> Y, then folded dense math on PE/ACT.
Bottleneck: Q7 SWDGE desc-gen ~5.4ns/desc -> minimize descriptors.
"""
import os
import sys

sys.path.insert(0, "/opt/trn_rl_repo")

import numpy as np

N, F, T, C, HOR = 50000, 4, 12, 32, 12
NCORES = 8
DPC = 6272
SC = DPC // 128        # 49 cols
NP = DPC * NCORES      # 50176
NH = NP // 2           # 25088 rows per half table
EW = 64                # 256B table row (48 real f32)

_prog_cache = {}


def _cfg():
    return dict(
        SP=os.environ.get("K_SINGLE_PACKET", "0") == "1",
        NQ=int(os.environ.get("K_NQUEUES", "4")),
        TARGB=int(os.environ.get("K_TARGB", "110")),  # target blocks/chunk
        SUBB=int(os.environ.get("K_SUBB", "20")),     # blocks per gather call
    )


def _build_program(KA, KB, chunks):
    """KA/KB: [49] per-col block counts. chunks: list of lists of col idxs."""
    from contextlib import ExitStack

    from concourse import bacc, mybir
    import concourse.tile as tile
    from concourse.masks import make_identity

    f32 = mybir.dt.float32
    bf16 = mybir.dt.bfloat16
    i16 = mybir.dt.int16
    AF = mybir.ActivationFunctionType
    ALU = mybir.AluOpType

    cfg = _cfg()
    SP, NQ = cfg["SP"], cfg["NQ"]

    SA = int(KA.sum())
    SB = int(KB.sum())
    # per-chunk block counts and offsets
    SA_c = [int(KA[cols].sum()) for cols in chunks]
    SB_c = [int(KB[cols].sum()) for cols in chunks]
    SAmax, SBmax = max(SA_c), max(SB_c)
    CHCmax = max(len(cols) for cols in chunks)

    nc = bacc.Bacc("TRN2", target_bir_lowering=False, debug=False,
                   num_swdge_queues=NQ)

    tabA = nc.dram_tensor("tabA", [NH, EW], f32, kind="ExternalInput")
    tabB = nc.dram_tensor("tabB", [NH, EW], f32, kind="ExternalInput")
    idxA_d = nc.dram_tensor("idxA", [128, SA * 8], i16, kind="ExternalInput")
    idxB_d = nc.dram_tensor("idxB", [128, SB * 8], i16, kind="ExternalInput")
    wA_d = nc.dram_tensor("wA", [128, SA], f32, kind="ExternalInput")
    wB_d = nc.dram_tensor("wB", [128, SB], f32, kind="ExternalInput")
    xselfw_d = nc.dram_tensor("xselfw", [128, SC * 48], f32, kind="ExternalInput")
    mz_d = nc.dram_tensor("mz3", [48, 3 * 128], bf16, kind="ExternalInput")
    mh_d = nc.dram_tensor("mh3", [48, 3 * 128], bf16, kind="ExternalInput")
    czv_d = nc.dram_tensor("czv", [128, 1], f32, kind="ExternalInput")
    chv_d = nc.dram_tensor("chv", [128, 1], f32, kind="ExternalInput")
    pm_d = nc.dram_tensor("pm3", [128, 3 * 32], bf16, kind="ExternalInput")
    wo_d = nc.dram_tensor("woutT", [32, HOR], bf16, kind="ExternalInput")
    bo_d = nc.dram_tensor("boutv", [HOR, 1], f32, kind="ExternalInput")
    out = nc.dram_tensor("out", [HOR, DPC], f32, kind="ExternalOutput")

    with tile.TileContext(nc) as tc, ExitStack() as ctx:
        pconst = ctx.enter_context(tc.tile_pool(name="pconst", bufs=1))
        pga = ctx.enter_context(tc.tile_pool(name="pga", bufs=3))
        pgb = ctx.enter_context(tc.tile_pool(name="pgb", bufs=3))
        ptma = ctx.enter_context(tc.tile_pool(name="ptma", bufs=2))
        ptmb = ctx.enter_context(tc.tile_pool(name="ptmb", bufs=2))
        pt = ctx.enter_context(tc.tile_pool(name="pt", bufs=2))
        pdense = ctx.enter_context(tc.tile_pool(name="pdense", bufs=2))
        pout = ctx.enter_context(tc.tile_pool(name="pout", bufs=2))
        pps_t = ctx.enter_context(tc.tile_pool(name="pps_t", bufs=2, space="PSUM"))
        pps_zh = ctx.enter_context(tc.tile_pool(name="pps_zh", bufs=2, space="PSUM"))
        pps_a = ctx.enter_context(tc.tile_pool(name="pps_a", bufs=1, space="PSUM"))
        pps_o = ctx.enter_context(tc.tile_pool(name="pps_o", bufs=1, space="PSUM"))

        # ---- resident constants -------------------------------------
        ident = pconst.tile([128, 128], f32)
        make_identity(nc, ident[:])
        # idx/w preloads are split per chunk (separate tags) so the first
        # gather depends only on its own chunk's small DMA, not the full
        # 1.9MB preload. Issue them in chunk-processing order (Sync FIFO).
        corder = sorted(range(len(chunks)), key=lambda ci: -len(chunks[ci]))
        offs_A = np.concatenate([[0], np.cumsum(SA_c)]).astype(int)
        offs_B = np.concatenate([[0], np.cumsum(SB_c)]).astype(int)
        idxA_ts, idxB_ts, wA_ts, wB_ts = {}, {}, {}, {}
        for ci in corder:
            sa, sb = SA_c[ci], SB_c[ci]
            oA, oB = int(offs_A[ci]), int(offs_B[ci])
            idxA_ts[ci] = pconst.tile([128, sa * 8], i16, tag=f"idxA{ci}", name=f"idxA{ci}")
            nc.sync.dma_start(idxA_ts[ci][:], idxA_d[:, oA * 8:(oA + sa) * 8])
            idxB_ts[ci] = pconst.tile([128, sb * 8], i16, tag=f"idxB{ci}", name=f"idxB{ci}")
            nc.sync.dma_start(idxB_ts[ci][:], idxB_d[:, oB * 8:(oB + sb) * 8])
            wA_ts[ci] = pconst.tile([128, sa], f32, tag=f"wA{ci}", name=f"wAc{ci}")
            nc.sync.dma_start(wA_ts[ci][:], wA_d[:, oA:oA + sa])
            wB_ts[ci] = pconst.tile([128, sb], f32, tag=f"wB{ci}", name=f"wBc{ci}")
            nc.sync.dma_start(wB_ts[ci][:], wB_d[:, oB:oB + sb])
        xsw_t = pconst.tile([128, SC, 48], f32, tag="xsw")
        nc.sync.dma_start(xsw_t[:], xselfw_d[:])
        mz_t = pconst.tile([48, 3 * 128], bf16, tag="mz")
        nc.sync.dma_start(mz_t[:], mz_d[:])
        mh_t = pconst.tile([48, 3 * 128], bf16, tag="mh")
        nc.sync.dma_start(mh_t[:], mh_d[:])
        cz_t = pconst.tile([128, 1], f32, tag="cz")
        nc.sync.dma_start(cz_t[:], czv_d[:])
        ch_t = pconst.tile([128, 1], f32, tag="ch")
        nc.sync.dma_start(ch_t[:], chv_d[:])
        pm_t = pconst.tile([128, 3 * 32], bf16, tag="pm")
        nc.sync.dma_start(pm_t[:], pm_d[:])
        wo_t = pconst.tile([32, HOR], bf16, tag="wo")
        nc.sync.dma_start(wo_t[:], wo_d[:])
        bo_t = pconst.tile([HOR, 1], f32, tag="bo")
        nc.sync.dma_start(bo_t[:], bo_d[:])

        qrr = 0
        # run the FEWEST-COLS chunk LAST to minimize the dense tail
        # after the final gather
        for ci in corder:
            cols = chunks[ci]
            chc = len(cols)
            sa, sb = SA_c[ci], SB_c[ci]
            node0 = cols[0] * 128
            # ---- gathers (the serial Q7 resource) -------------------
            # sub-calls round-robin the 4 SWDGE queues so desc-gen of
            # call k overlaps the ring drain of calls k-1..k-3
            SUBB = cfg["SUBB"]
            GA = pga.tile([128, SAmax, EW], f32, tag="GA")
            o = 0
            while o < sa:
                nb = min(SUBB, sa - o)
                nc.gpsimd.dma_gather(
                    GA[:, o:o + nb, :], tabA[:],
                    idxA_ts[ci][:, o * 8:(o + nb) * 8],
                    nb * 128, nb * 128, EW,
                    single_packet=SP, queue_num=qrr % NQ,
                )
                qrr += 1
                o += nb
            GB = pgb.tile([128, SBmax, EW], f32, tag="GB")
            o = 0
            while o < sb:
                nb = min(SUBB, sb - o)
                nc.gpsimd.dma_gather(
                    GB[:, o:o + nb, :], tabB[:],
                    idxB_ts[ci][:, o * 8:(o + nb) * 8],
                    nb * 128, nb * 128, EW,
                    single_packet=SP, queue_num=qrr % NQ,
                )
                qrr += 1
                o += nb
            # ---- weight multiply (f32 -> bf16 tmp) ------------------
            tmA = ptma.tile([128, SAmax, 48], bf16, tag="tmA")
            nc.vector.tensor_tensor(
                out=tmA[:, :sa, :], in0=GA[:, :sa, 0:48],
                in1=wA_ts[ci][:].to_broadcast([128, sa, 48]),
                op=ALU.mult,
            )
            tmB = ptmb.tile([128, SBmax, 48], bf16, tag="tmB")
            nc.vector.tensor_tensor(
                out=tmB[:, :sb, :], in0=GB[:, :sb, 0:48],
                in1=wB_ts[ci][:].to_broadcast([128, sb, 48]),
                op=ALU.mult,
            )
            # ---- per-col strided reduces (cols with equal K merged) --
            # reduce input AP: [128, cols(stride k*48), 48(stride 1),
            # k(stride 48)] -> reduce innermost axis
            def col_reduces(tm, tdst, Kvec):
                oj = 0
                jj = 0
                while jj < chc:
                    k = int(Kvec[cols[jj]])
                    ng = 1
                    while (jj + ng < chc and int(Kvec[cols[jj + ng]]) == k):
                        ng += 1
                    nc.vector.tensor_reduce(
                        out=tdst[:, jj:jj + ng, :],
                        in_=tm[:, oj:oj + ng * k, :]
                            .rearrange("p (g k) f -> p g f k", g=ng),
                        axis=mybir.AxisListType.X, op=ALU.add,
                    )
                    oj += ng * k
                    jj += ng
            tA = pt.tile([128, CHCmax, 48], f32, tag="tA")
            tB = pt.tile([128, CHCmax, 48], f32, tag="tB")
            col_reduces(tmA, tA, KA)
            col_reduces(tmB, tB, KB)
            Y = pt.tile([128, CHCmax, 48], f32, tag="Y")
            nc.vector.tensor_tensor(
                out=Y[:, :chc, :], in0=tA[:, :chc, :], in1=tB[:, :chc, :],
                op=ALU.add)
            nc.vector.tensor_tensor(
                out=Y[:, :chc, :], in0=Y[:, :chc, :],
                in1=xsw_t[:, cols[0]:cols[0] + chc, :], op=ALU.add)
            # ---- dense phase, sub-chunks of <=4 cols ----------------
            sub0 = 0
            while sub0 < chc:
                ncols = min(4, chc - sub0)
                nn = ncols * 128
                yt_ps = pps_t.tile([48, 512], f32, tag="ytp")
                for j in range(ncols):
                    nc.tensor.transpose(
                        out=yt_ps[:, j * 128:(j + 1) * 128],
                        in_=Y[:, sub0 + j, :],
                        identity=ident[:],
                    )
                yt = pdense.tile([48, 512], bf16, tag="yt")
                nc.vector.tensor_copy(out=yt[:, :nn], in_=yt_ps[:, :nn])
                acc_ps = pps_a.tile([32, 512], f32, tag="acc")
                for g in range(3):
                    z_ps = pps_zh.tile([128, 512], f32, tag="zps")
                    nc.tensor.matmul(
                        out=z_ps[:, :nn], lhsT=mz_t[:, g * 128:(g + 1) * 128],
                        rhs=yt[:, :nn], start=True, stop=True)
                    z_sb = pdense.tile([128, 512], bf16, tag="zsb")
                    nc.scalar.activation(z_sb[:, :nn], z_ps[:, :nn],
                                         AF.Sigmoid, bias=cz_t[:])
                    h_ps = pps_zh.tile([128, 512], f32, tag="hps")
                    nc.tensor.matmul(
                        out=h_ps[:, :nn], lhsT=mh_t[:, g * 128:(g + 1) * 128],
                        rhs=yt[:, :nn], start=True, stop=True)
                    h_sb = pdense.tile([128, 512], bf16, tag="hsb")
                    nc.scalar.activation(h_sb[:, :nn], h_ps[:, :nn],
                                         AF.Tanh, bias=ch_t[:])
                    zh = pdense.tile([128, 512], bf16, tag="zh")
                    nc.vector.tensor_tensor(out=zh[:, :nn], in0=z_sb[:, :nn],
                                            in1=h_sb[:, :nn], op=ALU.mult)
                    nc.tensor.matmul(
                        out=acc_ps[:, :nn], lhsT=pm_t[:, g * 32:(g + 1) * 32],
                        rhs=zh[:, :nn], start=(g == 0), stop=(g == 2))
                a_sb = pdense.tile([32, 512], bf16, tag="asb")
                nc.scalar.activation(a_sb[:, :nn], acc_ps[:, :nn], AF.Relu)
                o_ps = pps_o.tile([HOR, 512], f32, tag="ops")
                nc.tensor.matmul(out=o_ps[:, :nn], lhsT=wo_t[:],
                                 rhs=a_sb[:, :nn], start=True, stop=True)
                o_sb = pout.tile([HOR, 512], f32, tag="osb")
                nc.scalar.activation(o_sb[:, :nn], o_ps[:, :nn], AF.Identity,
                                     bias=bo_t[:])
                nc.sync.dma_start(out[:, node0:node0 + nn], o_sb[:, :nn])
                node0 += nn
                sub0 += ncols


    nc.compile()
    return nc


def _wrap16(lst):
    """idx position i -> [i%16 (+16g replicated), i//16]; lst len % 128 == 0"""
    arr = np.asarray(lst, np.int16).reshape(-1, 16).T  # [16, n/16]
    return np.tile(arr, (8, 1))                        # [128, n/16]


def prepare(x, edge_index, edge_weight, attention,
            Wz, bz, Wlz, blz, Wr, br, Wlr, blr, Wh, bh, Wlh, blh,
            Wout, bout):
    x = np.asarray(x, np.float32)
    edge_index = np.asarray(edge_index)
    ew = np.asarray(edge_weight, np.float32)
    attention = np.asarray(attention, np.float32)
    Wz, bz = np.asarray(Wz, np.float32), np.asarray(bz, np.float32)
    Wlz, blz = np.asarray(Wlz, np.float32), np.asarray(blz, np.float32)
    Wh, bh = np.asarray(Wh, np.float32), np.asarray(bh, np.float32)
    Wlh, blh = np.asarray(Wlh, np.float32), np.asarray(blh, np.float32)
    Wout, bout = np.asarray(Wout, np.float32), np.asarray(bout, np.float32)

    row = edge_index[0].astype(np.int64)
    col = edge_index[1].astype(np.int64)

    # ---- GCN norm (host: structure-only) ----------------------------
    deg = np.zeros(N, np.float32)
    np.add.at(deg, col, ew)
    deg += 1.0
    dis = 1.0 / np.sqrt(deg)
    norm = dis[row] * ew * dis[col]
    self_norm = np.zeros(NP, np.float32)
    self_norm[:N] = dis * dis

    # ---- X in [NP, 48] time-major, padded; halves -------------------
    X = np.zeros((NP, EW), np.float32)
    X[:N, :48] = x.transpose(0, 2, 1).reshape(N, 48)
    tabA = np.ascontiguousarray(X[:NH])
    tabB = np.ascontiguousarray(X[NH:])

    # ---- per-dest edge lists by half (CSR by col) -------------------
    half = (row >= NH).astype(np.int64)
    order = np.lexsort((half, col))
    r_s, c_s, n_s, h_s = row[order], col[order], norm[order], half[order]
    # boundaries per (col, half)
    cnt = np.zeros((NP, 2), np.int64)
    np.add.at(cnt, (c_s, h_s), 1)
    a_cnt, b_cnt = cnt[:, 0], cnt[:, 1]
    start = np.zeros(NP + 1, np.int64)
    np.cumsum(cnt.sum(1), out=start[1:])

    # ---- per-core ordering + per-col staircase shapes ---------------
    perms = []
    KA_all = np.zeros((NCORES, SC), np.int64)
    KB_all = np.zeros((NCORES, SC), np.int64)
    for ci in range(NCORES):
        lo = ci * DPC
        a = a_cnt[lo:lo + DPC]
        b = b_cnt[lo:lo + DPC]
        mx = np.maximum(a, b)
        mn = np.minimum(a, b)
        side = (a > b).astype(np.int64)
        perm = np.lexsort((-mn, -side, -mx))
        perms.append(perm)
        KA_all[ci] = a[perm].reshape(SC, 128).max(1)
        KB_all[ci] = b[perm].reshape(SC, 128).max(1)
    KA = KA_all.max(0)
    KB = KB_all.max(0)

    # ---- chunks balanced by block count -----------------------------
    targ = _cfg()["TARGB"]
    chunks, cur, acc = [], [], 0
    for j in range(SC):
        cur.append(j)
        acc += int(KA[j] + KB[j])
        if acc >= targ:
            chunks.append(cur)
            cur, acc = [], 0
    if cur:
        chunks.append(cur)
    SA, SB = int(KA.sum()), int(KB.sum())
    ndesc = (SA + SB) * 128
    print(f"v2: SA={SA} SB={SB} descs/core={ndesc} chunks={[len(c) for c in chunks]}")

    # ---- dense folded weights (negation folded into z path) ---------
    probs = np.exp(attention - attention.max())
    probs /= probs.sum()
    Mz = -(Wz @ Wlz[:C])
    cz = -(bz @ Wlz[:C] + blz)
    Mh = Wh @ Wlh[:C]
    ch = bh @ Wlh[:C] + blh
    mz3 = np.zeros((48, 3 * 128), np.float32)
    mh3 = np.zeros((48, 3 * 128), np.float32)
    pm3 = np.zeros((128, 3 * 32), np.float32)
    for t in range(T):
        g, tl = t // 4, t % 4
        mz3[t * 4:(t + 1) * 4, g * 128 + tl * 32:g * 128 + (tl + 1) * 32] = Mz
        mh3[t * 4:(t + 1) * 4, g * 128 + tl * 32:g * 128 + (tl + 1) * 32] = Mh
        pm3[tl * 32:(tl + 1) * 32, g * 32:(g + 1) * 32] = probs[t] * np.eye(32, dtype=np.float32)
    czv = np.tile(cz, 4).astype(np.float32)[:, None]
    chv = np.tile(ch, 4).astype(np.float32)[:, None]

    import ml_dtypes
    shared = {
        "tabA": tabA, "tabB": tabB,
        "mz3": mz3.astype(ml_dtypes.bfloat16),
        "mh3": mh3.astype(ml_dtypes.bfloat16),
        "czv": czv, "chv": chv,
        "pm3": pm3.astype(ml_dtypes.bfloat16),
        "woutT": Wout.astype(ml_dtypes.bfloat16),
        "boutv": bout.reshape(HOR, 1).astype(np.float32),
    }

    # ---- per-core idx/w tables following the chunk layout -----------
    in_maps = []
    for ci in range(NCORES):
        lo = ci * DPC
        perm = perms[ci]
        dests = lo + perm  # global dest ids, in (col, partition) order
        # per-dest edge slices (sorted run in r_s/n_s)
        d_start = start[dests]
        d_acnt = a_cnt[dests]
        d_bcnt = b_cnt[dests]
        idxA_list, wA_list = [], []
        idxB_list, wB_list = [], []
        for cols in chunks:
            for j in cols:
                k = int(KA[j])
                dj = slice(j * 128, (j + 1) * 128)
                st = d_start[dj]; ac = d_acnt[dj]
                idx_blk = np.zeros((k, 128), np.int16)
                w_blk = np.zeros((k, 128), np.float32)
                for p in range(128):
                    n_ = int(ac[p]); s_ = int(st[p])
                    idx_blk[:n_, p] = r_s[s_:s_ + n_]
                    w_blk[:n_, p] = n_s[s_:s_ + n_]
                idxA_list.append(idx_blk)
                wA_list.append(w_blk)
            for j in cols:
                k = int(KB[j])
                dj = slice(j * 128, (j + 1) * 128)
                st = d_start[dj]; ac = d_acnt[dj]; bc = d_bcnt[dj]
                idx_blk = np.zeros((k, 128), np.int16)
                w_blk = np.zeros((k, 128), np.float32)
                for p in range(128):
                    n_ = int(bc[p]); s_ = int(st[p]) + int(ac[p])
                    idx_blk[:n_, p] = r_s[s_:s_ + n_] - NH
                    w_blk[:n_, p] = n_s[s_:s_ + n_]
                idxB_list.append(idx_blk)
                wB_list.append(w_blk)
        # split back into A and B streams in chunk order
        # idxA_list currently interleaved per chunk: first len(cols) entries A,
        # then len(cols) B -- but we appended A to idxA_list and B to idxB_list
        # already in chunk order, so concatenation is correct.
        idxA_cat = np.concatenate([b.reshape(-1) for b in idxA_list])
        idxB_cat = np.concatenate([b.reshape(-1) for b in idxB_list])
        # weights: layout [128 partitions, blocks] where block-major matches
        # gather block order; w_blk is [k, 128] -> transpose to [128, k]
        wA_cat = np.concatenate([b.T for b in wA_list], axis=1)
        wB_cat = np.concatenate([b.T for b in wB_list], axis=1)
        xselfw = (self_norm[dests][:, None] * X[dests, :48]).astype(np.float32)
        xsw = xselfw.reshape(SC, 128, 48).transpose(1, 0, 2).reshape(128, SC * 48)
        m = dict(shared)
        m["idxA"] = np.ascontiguousarray(_wrap16(idxA_cat))
        m["idxB"] = np.ascontiguousarray(_wrap16(idxB_cat))
        m["wA"] = np.ascontiguousarray(wA_cat)
        m["wB"] = np.ascontiguousarray(wB_cat)
        m["xselfw"] = np.ascontiguousarray(xsw)
        in_maps.append(m)

    key = (tuple(KA), tuple(KB), tuple(len(c) for c in chunks),
           tuple(sorted(_cfg().items())))
    if key not in _prog_cache:
        _prog_cache[key] = _build_program(KA, KB, chunks)
    nc = _prog_cache[key]
    return nc, in_maps, perms


def kernel(**inputs):
    from concourse.bass_utils import run_bass_kernel_spmd

    nc, in_maps, perms = prepare(**inputs)
    res = run_bass_kernel_spmd(nc, in_maps, core_ids=list(range(NCORES)))
    full = np.zeros((NP, HOR), np.float32)
    for i in range(NCORES):
        full[i * DPC + perms[i]] = res.results[i]["out"].T
    return np.ascontiguousarray(full[:N]).astype(np.float32)


# revision 12
# speedup vs baseline: 1.1241x; 1.1241x over previous
"""A3TGCN v2: chunk-major staircase ELL, single mixed grid per half.

Math (H0 == 0 in the reference => R-gate dead, weights foldable):
  Y   = A_hat @ X          X = x as [N, 48] (time-major), one sparse agg
  1-Z = sigmoid(y @ -Mz - cz)        (negation folded into weights)
  H_t = tanh  (y @ Mh + ch)
  out = relu(sum_t p_t (1-Z_t) H_t) @ Wout + bout

Structure per core (6272 dests):
  dests sorted by (-max(a,b), -(a>b), -min(a,b)) -> cols of 128.
  Per col j: KA_j/KB_j = cross-core max #edges from table half A/B.
  Cols grouped into chunks balanced by block count. Per chunk: one
  dma_gather per half (col-major staircase layout), DVE mult into bf16
  tmp, per-col strided reduce -> Y, then folded dense math on PE/ACT.
Bottleneck: Q7 SWDGE desc-gen ~5.4ns/desc -> minimize descriptors.
"""
import os
import sys

sys.path.insert(0, "/opt/trn_rl_repo")

import numpy as np

N, F, T, C, HOR = 50000, 4, 12, 32, 12
NCORES = 8
DPC = 6272
SC = DPC // 128        # 49 cols
NP = DPC * NCORES      # 50176
NH = NP // 2           # 25088 rows per half table
EW = 64                # 256B table row (48 real f32)

_prog_cache = {}


def _cfg():
    return dict(
        SP=os.environ.get("K_SINGLE_PACKET", "0") == "1",
        NQ=int(os.environ.get("K_NQUEUES", "4")),
        TARGB=int(os.environ.get("K_TARGB", "110")),  # target blocks/chunk
        SUBB=int(os.environ.get("K_SUBB", "20")),     # blocks per gather call
    )


def _build_program(KA, KB, chunks):
    """KA/KB: [49] per-col block counts. chunks: list of lists of col idxs."""
    from contextlib import ExitStack

    from concourse import bacc, mybir
    import concourse.tile as tile
    from concourse.masks import make_identity

    f32 = mybir.dt.float32
    bf16 = mybir.dt.bfloat16
    i16 = mybir.dt.int16
    AF = mybir.ActivationFunctionType
    ALU = mybir.AluOpType

    cfg = _cfg()
    SP, NQ = cfg["SP"], cfg["NQ"]

    SA = int(KA.sum())
    SB = int(KB.sum())
    # per-chunk block counts and offsets
    SA_c = [int(KA[cols].sum()) for cols in chunks]
    SB_c = [int(KB[cols].sum()) for cols in chunks]
    SAmax, SBmax = max(SA_c), max(SB_c)
    CHCmax = max(len(cols) for cols in chunks)

    nc = bacc.Bacc("TRN2", target_bir_lowering=False, debug=False,
                   num_swdge_queues=NQ)

    tabA = nc.dram_tensor("tabA", [NH, EW], f32, kind="ExternalInput")
    tabB = nc.dram_tensor("tabB", [NH, EW], f32, kind="ExternalInput")
    idxA_d = nc.dram_tensor("idxA", [128, SA * 8], i16, kind="ExternalInput")
    idxB_d = nc.dram_tensor("idxB", [128, SB * 8], i16, kind="ExternalInput")
    wA_d = nc.dram_tensor("wA", [128, SA], f32, kind="ExternalInput")
    wB_d = nc.dram_tensor("wB", [128, SB], f32, kind="ExternalInput")
    xselfw_d = nc.dram_tensor("xselfw", [128, SC * 48], f32, kind="ExternalInput")
    mz_d = nc.dram_tensor("mz3", [48, 3 * 128], bf16, kind="ExternalInput")
    mh_d = nc.dram_tensor("mh3", [48, 3 * 128], bf16, kind="ExternalInput")
    czv_d = nc.dram_tensor("czv", [128, 1], f32, kind="ExternalInput")
    chv_d = nc.dram_tensor("chv", [128, 1], f32, kind="ExternalInput")
    pm_d = nc.dram_tensor("pm3", [128, 3 * 32], bf16, kind="ExternalInput")
    wo_d = nc.dram_tensor("woutT", [32, HOR], bf16, kind="ExternalInput")
    bo_d = nc.dram_tensor("boutv", [HOR, 1], f32, kind="ExternalInput")
    out = nc.dram_tensor("out", [HOR, DPC], f32, kind="ExternalOutput")

    with tile.TileContext(nc) as tc, ExitStack() as ctx:
        pconst = ctx.enter_context(tc.tile_pool(name="pconst", bufs=1))
        pga = ctx.enter_context(tc.tile_pool(name="pga", bufs=3))
        pgb = ctx.enter_context(tc.tile_pool(name="pgb", bufs=3))
        ptma = ctx.enter_context(tc.tile_pool(name="ptma", bufs=2))
        ptmb = ctx.enter_context(tc.tile_pool(name="ptmb", bufs=2))
        pt = ctx.enter_context(tc.tile_pool(name="pt", bufs=2))
        pdense = ctx.enter_context(tc.tile_pool(name="pdense", bufs=2))
        pout = ctx.enter_context(tc.tile_pool(name="pout", bufs=2))
        pps_t = ctx.enter_context(tc.tile_pool(name="pps_t", bufs=2, space="PSUM"))
        pps_zh = ctx.enter_context(tc.tile_pool(name="pps_zh", bufs=2, space="PSUM"))
        pps_a = ctx.enter_context(tc.tile_pool(name="pps_a", bufs=1, space="PSUM"))
        pps_o = ctx.enter_context(tc.tile_pool(name="pps_o", bufs=1, space="PSUM"))

        # ---- resident constants -------------------------------------
        ident = pconst.tile([128, 128], f32)
        make_identity(nc, ident[:])
        # idx/w preloads are split per chunk (separate tags) so the first
        # gather depends only on its own chunk's small DMA, not the full
        # 1.9MB preload. Issue them in chunk-processing order (Sync FIFO).
        corder = list(range(len(chunks)))
        offs_A = np.concatenate([[0], np.cumsum(SA_c)]).astype(int)
        offs_B = np.concatenate([[0], np.cumsum(SB_c)]).astype(int)
        idxA_ts, idxB_ts, wA_ts, wB_ts = {}, {}, {}, {}
        for ci in corder:
            sa, sb = SA_c[ci], SB_c[ci]
            oA, oB = int(offs_A[ci]), int(offs_B[ci])
            idxA_ts[ci] = pconst.tile([128, sa * 8], i16, tag=f"idxA{ci}", name=f"idxA{ci}")
            nc.sync.dma_start(idxA_ts[ci][:], idxA_d[:, oA * 8:(oA + sa) * 8])
            idxB_ts[ci] = pconst.tile([128, sb * 8], i16, tag=f"idxB{ci}", name=f"idxB{ci}")
            nc.sync.dma_start(idxB_ts[ci][:], idxB_d[:, oB * 8:(oB + sb) * 8])
            wA_ts[ci] = pconst.tile([128, sa], f32, tag=f"wA{ci}", name=f"wAc{ci}")
            nc.sync.dma_start(wA_ts[ci][:], wA_d[:, oA:oA + sa])
            wB_ts[ci] = pconst.tile([128, sb], f32, tag=f"wB{ci}", name=f"wBc{ci}")
            nc.sync.dma_start(wB_ts[ci][:], wB_d[:, oB:oB + sb])
        xsw_t = pconst.tile([128, SC, 48], f32, tag="xsw")
        nc.sync.dma_start(xsw_t[:], xselfw_d[:])
        mz_t = pconst.tile([48, 3 * 128], bf16, tag="mz")
        nc.sync.dma_start(mz_t[:], mz_d[:])
        mh_t = pconst.tile([48, 3 * 128], bf16, tag="mh")
        nc.sync.dma_start(mh_t[:], mh_d[:])
        cz_t = pconst.tile([128, 1], f32, tag="cz")
        nc.sync.dma_start(cz_t[:], czv_d[:])
        ch_t = pconst.tile([128, 1], f32, tag="ch")
        nc.sync.dma_start(ch_t[:], chv_d[:])
        pm_t = pconst.tile([128, 3 * 32], bf16, tag="pm")
        nc.sync.dma_start(pm_t[:], pm_d[:])
        wo_t = pconst.tile([32, HOR], bf16, tag="wo")
        nc.sync.dma_start(wo_t[:], wo_d[:])
        bo_t = pconst.tile([HOR, 1], f32, tag="bo")
        nc.sync.dma_start(bo_t[:], bo_d[:])

        qrr = 0
        # run the FEWEST-COLS chunk LAST to minimize the dense tail
        # after the final gather
        for ci in corder:
            cols = chunks[ci]
            chc = len(cols)
            sa, sb = SA_c[ci], SB_c[ci]
            node0 = cols[0] * 128
            # ---- gathers (the serial Q7 resource) -------------------
            # sub-calls round-robin the 4 SWDGE queues so desc-gen of
            # call k overlaps the ring drain of calls k-1..k-3
            SUBB = cfg["SUBB"]
            GA = pga.tile([128, SAmax, EW], f32, tag="GA")
            o = 0
            while o < sa:
                nb = min(SUBB, sa - o)
                nc.gpsimd.dma_gather(
                    GA[:, o:o + nb, :], tabA[:],
                    idxA_ts[ci][:, o * 8:(o + nb) * 8],
                    nb * 128, nb * 128, EW,
                    single_packet=SP, queue_num=qrr % NQ,
                )
                qrr += 1
                o += nb
            GB = pgb.tile([128, SBmax, EW], f32, tag="GB")
            o = 0
            while o < sb:
                nb = min(SUBB, sb - o)
                nc.gpsimd.dma_gather(
                    GB[:, o:o + nb, :], tabB[:],
                    idxB_ts[ci][:, o * 8:(o + nb) * 8],
                    nb * 128, nb * 128, EW,
                    single_packet=SP, queue_num=qrr % NQ,
                )
                qrr += 1
                o += nb
            # ---- weight multiply (f32 -> bf16 tmp) ------------------
            tmA = ptma.tile([128, SAmax, 48], bf16, tag="tmA")
            nc.vector.tensor_tensor(
                out=tmA[:, :sa, :], in0=GA[:, :sa, 0:48],
                in1=wA_ts[ci][:].to_broadcast([128, sa, 48]),
                op=ALU.mult,
            )
            tmB = ptmb.tile([128, SBmax, 48], bf16, tag="tmB")
            nc.vector.tensor_tensor(
                out=tmB[:, :sb, :], in0=GB[:, :sb, 0:48],
                in1=wB_ts[ci][:].to_broadcast([128, sb, 48]),
                op=ALU.mult,
            )
            # ---- per-col strided reduces (cols with equal K merged) --
            # reduce input AP: [128, cols(stride k*48), 48(stride 1),
            # k(stride 48)] -> reduce innermost axis
            def col_reduces(tm, tdst, Kvec):
                oj = 0
                jj = 0
                while jj < chc:
                    k = int(Kvec[cols[jj]])
                    ng = 1
                    while (jj + ng < chc and int(Kvec[cols[jj + ng]]) == k):
                        ng += 1
                    nc.vector.tensor_reduce(
                        out=tdst[:, jj:jj + ng, :],
                        in_=tm[:, oj:oj + ng * k, :]
                            .rearrange("p (g k) f -> p g f k", g=ng),
                        axis=mybir.AxisListType.X, op=ALU.add,
                    )
                    oj += ng * k
                    jj += ng
            tA = pt.tile([128, CHCmax, 48], f32, tag="tA")
            tB = pt.tile([128, CHCmax, 48], f32, tag="tB")
            col_reduces(tmA, tA, KA)
            col_reduces(tmB, tB, KB)
            Y = pt.tile([128, CHCmax, 48], f32, tag="Y")
            nc.vector.tensor_tensor(
                out=Y[:, :chc, :], in0=tA[:, :chc, :], in1=tB[:, :chc, :],
                op=ALU.add)
            nc.vector.tensor_tensor(
                out=Y[:, :chc, :], in0=Y[:, :chc, :],
                in1=xsw_t[:, cols[0]:cols[0] + chc, :], op=ALU.add)
            # ---- dense phase, sub-chunks of <=4 cols ----------------
            sub0 = 0
            while sub0 < chc:
                ncols = min(4, chc - sub0)
                nn = ncols * 128
                yt_ps = pps_t.tile([48, 512], f32, tag="ytp")
                for j in range(ncols):
                    nc.tensor.transpose(
                        out=yt_ps[:, j * 128:(j + 1) * 128],
                        in_=Y[:, sub0 + j, :],
                        identity=ident[:],
                    )
                yt = pdense.tile([48, 512], bf16, tag="yt")
                nc.vector.tensor_copy(out=yt[:, :nn], in_=yt_ps[:, :nn])
                acc_ps = pps_a.tile([32, 512], f32, tag="acc")
                for g in range(3):
                    z_ps = pps_zh.tile([128, 512], f32, tag="zps")
                    nc.tensor.matmul(
                        out=z_ps[:, :nn], lhsT=mz_t[:, g * 128:(g + 1) * 128],
                        rhs=yt[:, :nn], start=True, stop=True)
                    z_sb = pdense.tile([128, 512], bf16, tag="zsb")
                    nc.scalar.activation(z_sb[:, :nn], z_ps[:, :nn],
                                         AF.Sigmoid, bias=cz_t[:])
                    h_ps = pps_zh.tile([128, 512], f32, tag="hps")
                    nc.tensor.matmul(
                        out=h_ps[:, :nn], lhsT=mh_t[:, g * 128:(g + 1) * 128],
                        rhs=yt[:, :nn], start=True, stop=True)
                    h_sb = pdense.tile([128, 512], bf16, tag="hsb")
                    nc.scalar.activation(h_sb[:, :nn], h_ps[:, :nn],
                                         AF.Tanh, bias=ch_t[:])
                    zh = pdense.tile([128, 512], bf16, tag="zh")
                    nc.vector.tensor_tensor(out=zh[:, :nn], in0=z_sb[:, :nn],
                                            in1=h_sb[:, :nn], op=ALU.mult)
                    nc.tensor.matmul(
                        out=acc_ps[:, :nn], lhsT=pm_t[:, g * 32:(g + 1) * 32],
                        rhs=zh[:, :nn], start=(g == 0), stop=(g == 2))
                a_sb = pdense.tile([32, 512], bf16, tag="asb")
                nc.scalar.activation(a_sb[:, :nn], acc_ps[:, :nn], AF.Relu)
                o_ps = pps_o.tile([HOR, 512], f32, tag="ops")
                nc.tensor.matmul(out=o_ps[:, :nn], lhsT=wo_t[:],
                                 rhs=a_sb[:, :nn], start=True, stop=True)
                o_sb = pout.tile([HOR, 512], f32, tag="osb")
                nc.scalar.activation(o_sb[:, :nn], o_ps[:, :nn], AF.Identity,
                                     bias=bo_t[:])
                nc.sync.dma_start(out[:, node0:node0 + nn], o_sb[:, :nn])
                node0 += nn
                sub0 += ncols


    nc.compile()
    return nc


def _wrap16(lst):
    """idx position i -> [i%16 (+16g replicated), i//16]; lst len % 128 == 0"""
    arr = np.asarray(lst, np.int16).reshape(-1, 16).T  # [16, n/16]
    return np.tile(arr, (8, 1))                        # [128, n/16]


def prepare(x, edge_index, edge_weight, attention,
            Wz, bz, Wlz, blz, Wr, br, Wlr, blr, Wh, bh, Wlh, blh,
            Wout, bout):
    x = np.asarray(x, np.float32)
    edge_index = np.asarray(edge_index)
    ew = np.asarray(edge_weight, np.float32)
    attention = np.asarray(attention, np.float32)
    Wz, bz = np.asarray(Wz, np.float32), np.asarray(bz, np.float32)
    Wlz, blz = np.asarray(Wlz, np.float32), np.asarray(blz, np.float32)
    Wh, bh = np.asarray(Wh, np.float32), np.asarray(bh, np.float32)
    Wlh, blh = np.asarray(Wlh, np.float32), np.asarray(blh, np.float32)
    Wout, bout = np.asarray(Wout, np.float32), np.asarray(bout, np.float32)

    row = edge_index[0].astype(np.int64)
    col = edge_index[1].astype(np.int64)

    # ---- GCN norm (host: structure-only) ----------------------------
    deg = np.zeros(N, np.float32)
    np.add.at(deg, col, ew)
    deg += 1.0
    dis = 1.0 / np.sqrt(deg)
    norm = dis[row] * ew * dis[col]
    self_norm = np.zeros(NP, np.float32)
    self_norm[:N] = dis * dis

    # ---- X in [NP, 48] time-major, padded; halves -------------------
    X = np.zeros((NP, EW), np.float32)
    X[:N, :48] = x.transpose(0, 2, 1).reshape(N, 48)
    tabA = np.ascontiguousarray(X[:NH])
    tabB = np.ascontiguousarray(X[NH:])

    # ---- per-dest edge lists by half (CSR by col) -------------------
    half = (row >= NH).astype(np.int64)
    order = np.lexsort((half, col))
    r_s, c_s, n_s, h_s = row[order], col[order], norm[order], half[order]
    # boundaries per (col, half)
    cnt = np.zeros((NP, 2), np.int64)
    np.add.at(cnt, (c_s, h_s), 1)
    a_cnt, b_cnt = cnt[:, 0], cnt[:, 1]
    start = np.zeros(NP + 1, np.int64)
    np.cumsum(cnt.sum(1), out=start[1:])

    # ---- per-core ordering + per-col staircase shapes ---------------
    perms = []
    KA_all = np.zeros((NCORES, SC), np.int64)
    KB_all = np.zeros((NCORES, SC), np.int64)
    for ci in range(NCORES):
        lo = ci * DPC
        a = a_cnt[lo:lo + DPC]
        b = b_cnt[lo:lo + DPC]
        mx = np.maximum(a, b)
        mn = np.minimum(a, b)
        side = (a > b).astype(np.int64)
        perm = np.lexsort((-mn, -side, -mx))
        perms.append(perm)
        KA_all[ci] = a[perm].reshape(SC, 128).max(1)
        KB_all[ci] = b[perm].reshape(SC, 128).max(1)
    KA = KA_all.max(0)
    KB = KB_all.max(0)

    # ---- chunks balanced by block count -----------------------------
    targ = _cfg()["TARGB"]
    chunks, cur, acc = [], [], 0
    for j in range(SC):
        cur.append(j)
        acc += int(KA[j] + KB[j])
        if acc >= targ:
            chunks.append(cur)
            cur, acc = [], 0
    if cur:
        chunks.append(cur)
    SA, SB = int(KA.sum()), int(KB.sum())
    ndesc = (SA + SB) * 128
    print(f"v2: SA={SA} SB={SB} descs/core={ndesc} chunks={[len(c) for c in chunks]}")

    # ---- dense folded weights (negation folded into z path) ---------
    probs = np.exp(attention - attention.max())
    probs /= probs.sum()
    Mz = -(Wz @ Wlz[:C])
    cz = -(bz @ Wlz[:C] + blz)
    Mh = Wh @ Wlh[:C]
    ch = bh @ Wlh[:C] + blh
    mz3 = np.zeros((48, 3 * 128), np.float32)
    mh3 = np.zeros((48, 3 * 128), np.float32)
    pm3 = np.zeros((128, 3 * 32), np.float32)
    for t in range(T):
        g, tl = t // 4, t % 4
        mz3[t * 4:(t + 1) * 4, g * 128 + tl * 32:g * 128 + (tl + 1) * 32] = Mz
        mh3[t * 4:(t + 1) * 4, g * 128 + tl * 32:g * 128 + (tl + 1) * 32] = Mh
        pm3[tl * 32:(tl + 1) * 32, g * 32:(g + 1) * 32] = probs[t] * np.eye(32, dtype=np.float32)
    czv = np.tile(cz, 4).astype(np.float32)[:, None]
    chv = np.tile(ch, 4).astype(np.float32)[:, None]

    import ml_dtypes
    shared = {
        "tabA": tabA, "tabB": tabB,
        "mz3": mz3.astype(ml_dtypes.bfloat16),
        "mh3": mh3.astype(ml_dtypes.bfloat16),
        "czv": czv, "chv": chv,
        "pm3": pm3.astype(ml_dtypes.bfloat16),
        "woutT": Wout.astype(ml_dtypes.bfloat16),
        "boutv": bout.reshape(HOR, 1).astype(np.float32),
    }

    # ---- per-core idx/w tables following the chunk layout -----------
    in_maps = []
    for ci in range(NCORES):
        lo = ci * DPC
        perm = perms[ci]
        dests = lo + perm  # global dest ids, in (col, partition) order
        # per-dest edge slices (sorted run in r_s/n_s)
        d_start = start[dests]
        d_acnt = a_cnt[dests]
        d_bcnt = b_cnt[dests]
        idxA_list, wA_list = [], []
        idxB_list, wB_list = [], []
        for cols in chunks:
            for j in cols:
                k = int(KA[j])
                dj = slice(j * 128, (j + 1) * 128)
                st = d_start[dj]; ac = d_acnt[dj]
                idx_blk = np.zeros((k, 128), np.int16)
                w_blk = np.zeros((k, 128), np.float32)
                for p in range(128):
                    n_ = int(ac[p]); s_ = int(st[p])
                    idx_blk[:n_, p] = r_s[s_:s_ + n_]
                    w_blk[:n_, p] = n_s[s_:s_ + n_]
                idxA_list.append(idx_blk)
                wA_list.append(w_blk)
            for j in cols:
                k = int(KB[j])
                dj = slice(j * 128, (j + 1) * 128)
                st = d_start[dj]; ac = d_acnt[dj]; bc = d_bcnt[dj]
                idx_blk = np.zeros((k, 128), np.int16)
                w_blk = np.zeros((k, 128), np.float32)
                for p in range(128):
                    n_ = int(bc[p]); s_ = int(st[p]) + int(ac[p])
                    idx_blk[:n_, p] = r_s[s_:s_ + n_] - NH
                    w_blk[:n_, p] = n_s[s_:s_ + n_]
                idxB_list.append(idx_blk)
                wB_list.append(w_blk)
        # split back into A and B streams in chunk order
        # idxA_list currently interleaved per chunk: first len(cols) entries A,
        # then len(cols) B -- but we appended A to idxA_list and B to idxB_list
        # already in chunk order, so concatenation is correct.
        idxA_cat = np.concatenate([b.reshape(-1) for b in idxA_list])
        idxB_cat = np.concatenate([b.reshape(-1) for b in idxB_list])
        # weights: layout [128 partitions, blocks] where block-major matches
        # gather block order; w_blk is [k, 128] -> transpose to [128, k]
        wA_cat = np.concatenate([b.T for b in wA_list], axis=1)
        wB_cat = np.concatenate([b.T for b in wB_list], axis=1)
        xselfw = (self_norm[dests][:, None] * X[dests, :48]).astype(np.float32)
        xsw = xselfw.reshape(SC, 128, 48).transpose(1, 0, 2).reshape(128, SC * 48)
        m = dict(shared)
        m["idxA"] = np.ascontiguousarray(_wrap16(idxA_cat))
        m["idxB"] = np.ascontiguousarray(_wrap16(idxB_cat))
        m["wA"] = np.ascontiguousarray(wA_cat)
        m["wB"] = np.ascontiguousarray(wB_cat)
        m["xselfw"] = np.ascontiguousarray(xsw)
        in_maps.append(m)

    key = (tuple(KA), tuple(KB), tuple(len(c) for c in chunks),
           tuple(sorted(_cfg().items())))
    if key not in _prog_cache:
        _prog_cache[key] = _build_program(KA, KB, chunks)
    nc = _prog_cache[key]
    return nc, in_maps, perms


def kernel(**inputs):
    from concourse.bass_utils import run_bass_kernel_spmd

    nc, in_maps, perms = prepare(**inputs)
    res = run_bass_kernel_spmd(nc, in_maps, core_ids=list(range(NCORES)))
    full = np.zeros((NP, HOR), np.float32)
    for i in range(NCORES):
        full[i * DPC + perms[i]] = res.results[i]["out"].T
    return np.ascontiguousarray(full[:N]).astype(np.float32)
